# revision 17
# baseline (speedup 1.0000x reference)
"""Trainium2 Bass kernel for the 4-kernel MMD permutation test (nn_DUAL_78237124264373).

Device does the O(768^2) work; host does the O(768) work.

Per core (25 of the 200 permutations; everything else replicated):
  Z = [X; Y] (768 x 64) is bf16-rounded on host.  d2 row-tiles come from one
  rank-66 bf16 matmul  d2 = L^T R  with L = [Zt; 1; 1], R = [-2 Zt; sq_hi;
  sq_lo]  (sq split hi+lo keeps the diagonal residual ~1e-3).  The ACT sqrt
  reads each PSUM tile directly with per-partition bias sq_r + B (B=1e-2
  keeps the diagonal sqrt finite; host constants account for it).  After ONE
  Exp table switch (the Sqrt load hides in the DMA wait) each kernel matrix
  K0_k is a single wide 4608-col exp into bf16, immediately consumed by
  M0 = A_aug K0 (col-tiled: kernel k's rows land at partitions 32k+i).
  Device ships back q0 = rowwise <M0, A_aug> plus M0's 1_X K0 / 1_Y K0 rows;
  the host assembles U_b = KAP*(q0 - arow) + W_corr @ e + (2/C2) t + ck and
  the U column from those sums and its own exact pair/stripe exps.
"""

import sys

import numpy as np

if "/opt/trn_rl_repo" not in sys.path:
    sys.path.insert(0, "/opt/trn_rl_repo")

import concourse.bacc as bacc
import concourse.bass as bass
import concourse.mybir as mybir
import concourse.tile as tile
from concourse import bass_utils

N = 384
NM = 768
D = 64
NPER = 200
NC = 8
PPC = NPER // NC  # 25
C1 = float(N * (N - 1))
C2 = float(N * N)
KAP = 2.0 / C1 + 2.0 / C2
CB1 = 1.0 / C1 + 2.0 / C2
CB2 = 1.0 / C1
TCO = 2.0 / C2
IC1 = 1.0 / C1
IC2 = 1.0 / C2
BIAS = 0.01  # added under the laplacian sqrt; host terms match
KERNELS = ("gaussian", "laplacian", "gaussian", "laplacian")

F32 = mybir.dt.float32
BF16 = mybir.dt.bfloat16
AF = mybir.ActivationFunctionType
ALU = mybir.AluOpType

W_AUXA = 12   # sqcB 0:6, per-kernel act scales 6:10


def _build():
    nc = bacc.Bacc("TRN2", target_bir_lowering=False, debug=False)
    with tile.TileContext(nc) as tc:
        with tc.tile_pool(name="dram", bufs=1, space="DRAM") as dram, \
             tc.tile_pool(name="io", bufs=1) as io, \
             tc.tile_pool(name="big", bufs=1) as big, \
             tc.tile_pool(name="kpool", bufs=4) as kpool, \
             tc.tile_pool(name="scr", bufs=1) as scr, \
             tc.tile_pool(name="sml", bufs=1) as sml:

            def din(name, shape, dt=F32):
                return dram.tile(shape, dt, kind="ExternalInput", name=name,
                                 uniquify=False)

            lrz_d = din("lrz", [66, 2 * NM], BF16)
            atp_d = din("atp", [128, 192], BF16)
            astk_d = din("astk", [128, NM + 128 + 16])
            rs_d = dram.tile([8, NM], F32, kind="ExternalOutput",
                             name="rs", uniquify=False)
            q0_d = dram.tile([1, 128], F32, kind="ExternalOutput",
                             name="q0", uniquify=False)

            # ---- input DMAs ----
            lrz = io.tile([66, 2 * NM], BF16, name="lrz_sb")
            atp = io.tile([128, 192], BF16, name="atp_sb")
            astk = io.tile([128, NM + 128 + 16], F32, name="astk_sb")
            nc.sync.dma_start(out=lrz[:, NM:], in_=lrz_d[:, NM:])
            nc.scalar.dma_start(out=lrz[:, 0:NM], in_=lrz_d[:, 0:NM])
            nc.sync.dma_start(out=astk[:, NM + 128:], in_=astk_d[:, NM + 128:])
            nc.sync.dma_start(out=atp[:], in_=atp_d[:])
            nc.sync.dma_start(out=astk[:, 0:NM + 128],
                              in_=astk_d[:, 0:NM + 128])

            L = lrz[:, 0:NM]
            R = lrz[:, NM:2 * NM]
            ident = astk[:, NM:NM + 128]
            sqcB = astk[:, NM + 128:NM + 134]
            scl = astk[:, NM + 134:NM + 138]

            d2sb = big.tile([128, 6 * NM], F32, name="d2sb")
            dist = big.tile([128, 6 * NM], F32, name="dist_sb")
            ones = io.tile([128, 1], F32, name="ones_sb")
            nc.vector.memset(ones[:], 1.0)

            # Dummy sqrt: hides the Sqrt ACT-table load in the DMA wait.
            dummy = sml.tile([1, 1], F32, name="dummy")
            nc.scalar.activation(dummy[:], ones[0:1, 0:1], AF.Sqrt)

            kts = [kpool.tile([128, 6 * NM], BF16, name=f"kt{k}",
                              tag="kt") for k in range(4)]

            # ---- phase 1: d2 = L^T R per row-tile (4 rotating PSUM bufs);
            # ACT sqrt straight out of PSUM with bias sq_r + B, DVE adds the
            # same bias into f32 d2sb for the gaussian exps ----
            with tc.tile_pool(name="psA", bufs=4, space="PSUM") as psA:
                for r in range(6):
                    ps = psA.tile([128, NM], F32, name=f"d2t{r}", tag="d2")
                    lhs = L[:, 128 * r:128 * (r + 1)]
                    nc.tensor.matmul(ps[:, 0:512], lhs, R[:, 0:512],
                                     start=True, stop=True)
                    nc.tensor.matmul(ps[:, 512:768], lhs, R[:, 512:768],
                                     start=True, stop=True)
                    sl = slice(NM * r, NM * (r + 1))
                    nc.scalar.activation(dist[:, sl], ps[:], AF.Sqrt,
                                         bias=sqcB[:, r:r + 1])
                    nc.vector.tensor_scalar(
                        out=d2sb[:, sl], in0=ps[:],
                        scalar1=sqcB[:, r:r + 1], scalar2=0.0,
                        op0=ALU.add, op1=ALU.add)

            q0c = sml.tile([128, 1], F32, name="q0c")

            with tc.tile_pool(name="psB", bufs=1, space="PSUM") as psB:
                ps_m = psB.tile([128, NM], F32, name="ps_m")

                # ---- phase 2: one wide exp per kernel, then its M0 ----
                def m0_chunks(k, cs):
                    pr = slice(32 * k, 32 * k + 32)
                    for c in cs:
                        lhs = atp[:, 32 * c:32 * (c + 1)]
                        nc.tensor.matmul(ps_m[pr, 0:512], lhs,
                                         kts[k][:, NM * c:NM * c + 512],
                                         start=(c == 0), stop=(c == 5),
                                         tile_position=(0, 32 * k),
                                         skip_group_check=True)
                        nc.tensor.matmul(ps_m[pr, 512:NM], lhs,
                                         kts[k][:, NM * c + 512:NM * (c + 1)],
                                         start=(c == 0), stop=(c == 5),
                                         tile_position=(0, 32 * k),
                                         skip_group_check=True)

                for k in (1, 3, 0):
                    src = d2sb if KERNELS[k] == "gaussian" else dist
                    nc.scalar.activation(kts[k][:], src[:], AF.Exp,
                                         scale=scl[:, k:k + 1])
                    m0_chunks(k, range(6))
                # last kernel in halves so M0 overlaps the second exp
                nc.scalar.activation(kts[2][:, 0:3 * NM], d2sb[:, 0:3 * NM],
                                     AF.Exp, scale=scl[:, 2:3])
                m0_chunks(2, range(3))
                nc.scalar.activation(kts[2][:, 3 * NM:6 * NM],
                                     d2sb[:, 3 * NM:6 * NM],
                                     AF.Exp, scale=scl[:, 2:3])
                m0_chunks(2, range(3, 6))

                # ---- phase 3: q0 + ship row stats; host does the rest.
                # astk rows 32k+25/26 are all-ones, so sC rows 25/26 are the
                # unmasked 1_X K0 / 1_Y K0 rows and DMA straight out. ----
                sC = scr.tile([128, NM], F32, name="sC", tag="sC")
                nc.vector.scalar_tensor_tensor(
                    out=sC[:], in0=ps_m[:], scalar=1.0, in1=astk[:, 0:NM],
                    op0=ALU.mult, op1=ALU.mult, accum_out=q0c[:])

                for k in range(4):
                    eng = nc.sync if k < 2 else nc.scalar
                    eng.dma_start(out=rs_d[2 * k:2 * k + 2, :],
                                  in_=sC[32 * k + 25:32 * k + 27, 0:NM])
                ps_q = psB.tile([1, 128], F32, name="ps_q")
                nc.tensor.transpose(ps_q[0:1, 0:128], q0c[:, 0:1], ident)
                q0row = sml.tile([1, 128], F32, name="q0row")
                nc.vector.tensor_copy(q0row[:], ps_q[:])
                nc.scalar.dma_start(out=q0_d[:], in_=q0row[:])

    nc.compile()
    return nc


def _host_prep(X, Y, bandwidths, perms):
    import ml_dtypes

    X = np.ascontiguousarray(X, np.float32)
    Y = np.ascontiguousarray(Y, np.float32)
    perms = np.ascontiguousarray(perms, np.int32)
    Zbf = np.concatenate([X, Y], 0).astype(ml_dtypes.bfloat16)
    Z = Zbf.astype(np.float32)                      # device-visible values
    sq = np.einsum("ij,ij->i", Z, Z, dtype=np.float32).astype(np.float32)
    sq_hi = sq.astype(ml_dtypes.bfloat16)
    sq_lo = (sq - sq_hi.astype(np.float32)).astype(ml_dtypes.bfloat16)

    lrz = np.zeros((66, 2 * NM), ml_dtypes.bfloat16)
    lrz[0:D, 0:NM] = Z.T
    lrz[D:66, 0:NM] = 1.0
    lrz[0:D, NM:] = (-2.0 * Z.T).astype(ml_dtypes.bfloat16)
    lrz[D, NM:] = sq_hi
    lrz[D + 1, NM:] = sq_lo

    b = np.asarray(bandwidths, np.float64)
    scl = np.zeros(4, np.float32)
    for k, kern in enumerate(KERNELS):
        scl[k] = -1.0 / (b[k] * b[k]) if kern == "gaussian" else -1.0 / b[k]

    extra = np.zeros((128, 128 + 16), np.float32)
    extra[:, 0:128] = np.eye(128, dtype=np.float32)
    for r in range(6):
        extra[:, 128 + r] = sq[128 * r:128 * (r + 1)] + np.float32(BIAS)
    extra[:, 134:138] = scl[None, :]

    maps = []
    for cid in range(NC):
        pm = perms[cid * PPC:(cid + 1) * PPC]
        A = np.zeros((27, NM), np.float32)
        A[np.arange(PPC)[:, None], pm[:, :N]] = 1
        A[25, :N] = 1
        A[26, N:] = 1
        astk = np.zeros((128, NM), np.float32)
        for k in range(4):
            astk[32 * k:32 * k + 27] = A
            astk[32 * k + 25:32 * k + 27] = 1.0  # rows 25/26: ship full rows
        atp = np.zeros((128, 6 * 32), np.float32)
        for c in range(6):
            atp[:, 32 * c:32 * c + 27] = A[:, 128 * c:128 * (c + 1)].T
        maps.append(dict(lrz=lrz, atp=atp.astype(ml_dtypes.bfloat16),
                         astk=np.concatenate([astk, extra], 1)))
    return maps


def _postprocess(results, X, Y, bandwidths, perms):
    """Assemble [4, 1+200] from per-core (rsx, rsy, q0) + exact host terms."""
    import ml_dtypes

    X = np.ascontiguousarray(X, np.float32)
    Y = np.ascontiguousarray(Y, np.float32)
    perms = np.asarray(perms, np.int64)
    Z = np.concatenate([X, Y], 0).astype(ml_dtypes.bfloat16).astype(
        np.float64)
    sq = (Z * Z).sum(1)
    b = np.asarray(bandwidths, np.float64)
    j = np.arange(N)
    d2e = sq[j] + sq[N + j] - 2.0 * np.einsum("ij,ij->i", Z[j], Z[N + j])

    def kexp(k, d2):
        d2 = d2 + BIAS
        if KERNELS[k] == "gaussian":
            return np.exp(-d2 / (b[k] * b[k]))
        return np.exp(-np.sqrt(d2) / b[k])

    full = np.zeros((4, 1 + NPER), np.float64)
    for cid in range(NC):
        res = results[cid]
        rsb = res["rs"].astype(np.float64).reshape(4, 2, NM)
        rsx, rsy = rsb[:, 0], rsb[:, 1]       # 1_X K0, 1_Y K0 per kernel
        q0s = res["q0"].astype(np.float64).reshape(128)
        pm = perms[cid * PPC:(cid + 1) * PPC]
        pX, pY = pm[:, :N], pm[:, N:]
        d2p = (sq[pX] + sq[pY]
               - 2.0 * np.einsum("pij,pij->pi", Z[pX], Z[pY]))  # [25, 384]
        stripe = pY == pX + N
        Am = np.zeros((PPC, NM))
        Am[np.arange(PPC)[:, None], pX] = 1.0
        A1, A2 = Am[:, :N], Am[:, N:]
        Wc = -KAP * (A1 * A2) + CB1 * A1 + CB2 * A2
        for k in range(4):
            rs = rsx[k] + rsy[k]
            arow = rs[pX].sum(1)
            q0 = q0s[32 * k:32 * k + PPC]
            ek = kexp(k, d2e)
            Kp = kexp(k, d2p)
            Kp[stripe] = 0.0
            t = Kp.sum(1)
            corr = Wc @ ek
            XX = rsx[k, :N].sum()
            YX = rsy[k, :N].sum()
            XY0 = rsx[k, N:].sum()
            YY = rsy[k, N:].sum()
            S = XX + YX + XY0 + YY
            sev = ek.sum()
            if KERNELS[k] == "gaussian":
                d0c = np.exp(-BIAS / (b[k] * b[k]))
            else:
                d0c = np.exp(-np.sqrt(BIAS) / b[k])
            tr = 768.0 * d0c
            ck = (S - sev - tr) * IC1
            ub = KAP * (q0 - arow) + corr + TCO * t + ck
            full[k, 1 + cid * PPC:1 + (cid + 1) * PPC] = ub
            if cid == 0:
                full[k, 0] = ((XX + YY - tr) * IC1
                              - 2.0 * IC2 * (XY0 - sev))
    return full.astype(np.float32)


_NC_CACHE = None


def _get_nc():
    global _NC_CACHE
    if _NC_CACHE is None:
        _NC_CACHE = _build()
    return _NC_CACHE


def kernel(X, Y, bandwidths, perms):
    nc = _get_nc()
    in_maps = _host_prep(X, Y, bandwidths, perms)
    res = bass_utils.run_bass_kernel_spmd(nc, in_maps, list(range(NC)))
    return _postprocess(res.results, X, Y, bandwidths, perms)


# revision 20
# speedup vs baseline: 1.4664x; 1.4664x over previous
"""Trainium2 Bass kernel for the 4-kernel MMD permutation test (nn_DUAL_78237124264373).

Sharding: 8 cores x (kernel-pair, perm-block).  Cores 0-3 compute kernels
{0 gaussian, 1 laplacian} for perms 50c..50c+49; cores 4-7 compute kernels
{2, 3} likewise.  Every core runs the SAME program: one wide exp over d2sb
(gaussian) + one over dist (laplacian), so exp work per core is 2x4608
columns instead of 4x.  Device does the O(768^2) work; host the O(768) work.

Per core:
  Z = [X; Y] (768 x 64) is bf16-rounded on host.  d2 row-tiles come from one
  rank-66 bf16 matmul  d2 = L^T R  with L = [Zt; 1; 1], R = [-2 Zt; sq_hi;
  sq_lo]  (sq split hi+lo keeps the diagonal residual ~1e-3).  The ACT sqrt
  reads each PSUM tile directly with per-partition bias sq_r + B (B=1e-2
  keeps the diagonal sqrt finite; host constants match).  After ONE Exp
  table switch (the Sqrt load hides in the DMA wait) each kernel matrix K0
  is a single wide 4608-col bf16 exp, immediately consumed by M0 = A_aug K0
  (A_aug = 50 perm indicators + 1_X + 1_Y, group h at partitions 64h+i).
  One DVE pass makes sC = M0 o A_aug with rowwise accum q0 = a K0 a into
  sC[:, 768]; astk rows 64h+{50,51} are all-ones so those sC rows are the
  unmasked 1_X K0 / 1_Y K0 row sums.  ONE DMA ships sC; the host assembles
  U_b = KAP*(q0 - arow) + W_corr @ e + (2/C2) t + ck and the U column from
  those sums and its own exact pair/stripe exps.
"""

import sys

import numpy as np

if "/opt/trn_rl_repo" not in sys.path:
    sys.path.insert(0, "/opt/trn_rl_repo")

import concourse.bacc as bacc
import concourse.bass as bass
import concourse.mybir as mybir
import concourse.tile as tile
from concourse import bass_utils

N = 384
NM = 768
D = 64
NPER = 200
NC = 8
CPP = 50   # perms per core (4 perm-blocks x 2 kernel-pairs)
C1 = float(N * (N - 1))
C2 = float(N * N)
KAP = 2.0 / C1 + 2.0 / C2
CB1 = 1.0 / C1 + 2.0 / C2
CB2 = 1.0 / C1
TCO = 2.0 / C2
IC1 = 1.0 / C1
IC2 = 1.0 / C2
BIAS = 0.01  # added under the laplacian sqrt; host terms match
KERNELS = ("gaussian", "laplacian", "gaussian", "laplacian")

F32 = mybir.dt.float32
BF16 = mybir.dt.bfloat16
AF = mybir.ActivationFunctionType
ALU = mybir.AluOpType

W_SC = 769  # sC: masked M0 cols 0:768, q0 accum col 768


def _build():
    nc = bacc.Bacc("TRN2", target_bir_lowering=False, debug=False)
    with tile.TileContext(nc) as tc:
        with tc.tile_pool(name="dram", bufs=1, space="DRAM") as dram, \
             tc.tile_pool(name="io", bufs=1) as io, \
             tc.tile_pool(name="big", bufs=1) as big, \
             tc.tile_pool(name="kpool", bufs=2) as kpool, \
             tc.tile_pool(name="scr", bufs=1) as scr, \
             tc.tile_pool(name="sml", bufs=1) as sml:

            def din(name, shape, dt=F32):
                return dram.tile(shape, dt, kind="ExternalInput", name=name,
                                 uniquify=False)

            lrz_d = din("lrz", [66, 2 * NM], BF16)
            atp_d = din("atp", [128, 6 * 64], BF16)
            astk_d = din("astk", [128, NM + 16])
            sc_d = dram.tile([128, W_SC], F32, kind="ExternalOutput",
                             name="sc", uniquify=False)

            # ---- input DMAs (all on sync; lrz first) ----
            lrz = io.tile([66, 2 * NM], BF16, name="lrz_sb")
            atp = io.tile([128, 6 * 64], BF16, name="atp_sb")
            astk = io.tile([128, NM + 16], F32, name="astk_sb")
            nc.sync.dma_start(out=lrz[:], in_=lrz_d[:])
            nc.sync.dma_start(out=astk[:, NM:], in_=astk_d[:, NM:])
            nc.sync.dma_start(out=atp[:], in_=atp_d[:])
            nc.sync.dma_start(out=astk[:, 0:NM], in_=astk_d[:, 0:NM])

            L = lrz[:, 0:NM]
            R = lrz[:, NM:2 * NM]
            sqcB = astk[:, NM:NM + 6]
            scl = astk[:, NM + 6:NM + 8]   # col 0: gaussian, col 1: laplacian

            d2sb = big.tile([128, 6 * NM], F32, name="d2sb")
            dist = big.tile([128, 6 * NM], F32, name="dist_sb")
            ones = io.tile([128, 1], F32, name="ones_sb")
            nc.vector.memset(ones[:], 1.0)

            # Dummy sqrt: hides the Sqrt ACT-table load in the DMA wait.
            dummy = sml.tile([1, 1], F32, name="dummy")
            nc.scalar.activation(dummy[:], ones[0:1, 0:1], AF.Sqrt)

            kts = [kpool.tile([128, 6 * NM], BF16, name=f"kt{h}",
                              tag="kt") for h in range(2)]

            # ---- phase 1: d2 = L^T R per row-tile (4 rotating PSUM bufs);
            # ACT sqrt straight out of PSUM with bias sq_r + B, DVE adds the
            # same bias into f32 d2sb for the gaussian exp ----
            with tc.tile_pool(name="psA", bufs=4, space="PSUM") as psA:
                for r in range(6):
                    ps = psA.tile([128, NM], F32, name=f"d2t{r}", tag="d2")
                    lhs = L[:, 128 * r:128 * (r + 1)]
                    nc.tensor.matmul(ps[:, 0:512], lhs, R[:, 0:512],
                                     start=True, stop=True)
                    nc.tensor.matmul(ps[:, 512:768], lhs, R[:, 512:768],
                                     start=True, stop=True)
                    sl = slice(NM * r, NM * (r + 1))
                    nc.scalar.activation(dist[:, sl], ps[:], AF.Sqrt,
                                         bias=sqcB[:, r:r + 1])
                    nc.vector.tensor_scalar(
                        out=d2sb[:, sl], in0=ps[:],
                        scalar1=sqcB[:, r:r + 1], scalar2=0.0,
                        op0=ALU.add, op1=ALU.add)

            with tc.tile_pool(name="psB", bufs=1, space="PSUM") as psB:
                ps_m = psB.tile([128, NM], F32, name="ps_m")

                # ---- phase 2: one wide exp per kernel, then its M0 ----
                def m0_chunks(h, cs):
                    pr = slice(64 * h, 64 * h + 64)
                    for c in cs:
                        lhs = atp[:, 64 * c:64 * (c + 1)]
                        nc.tensor.matmul(ps_m[pr, 0:512], lhs,
                                         kts[h][:, NM * c:NM * c + 512],
                                         start=(c == 0), stop=(c == 5),
                                         tile_position=(0, 64 * h),
                                         skip_group_check=True)
                        nc.tensor.matmul(ps_m[pr, 512:NM], lhs,
                                         kts[h][:, NM * c + 512:NM * (c + 1)],
                                         start=(c == 0), stop=(c == 5),
                                         tile_position=(0, 64 * h),
                                         skip_group_check=True)

                # laplacian first (dist is ready right after the sqrts);
                # gaussian in halves so its M0 overlaps the second half
                nc.scalar.activation(kts[1][:], dist[:], AF.Exp,
                                     scale=scl[:, 1:2])
                m0_chunks(1, range(6))
                nc.scalar.activation(kts[0][:, 0:3 * NM], d2sb[:, 0:3 * NM],
                                     AF.Exp, scale=scl[:, 0:1])
                m0_chunks(0, range(3))
                nc.scalar.activation(kts[0][:, 3 * NM:6 * NM],
                                     d2sb[:, 3 * NM:6 * NM],
                                     AF.Exp, scale=scl[:, 0:1])
                m0_chunks(0, range(3, 6))

                # ---- phase 3: sC = M0 o A_aug with q0 accum into col 768;
                # astk rows 64h+{50,51} are ones so those rows ship the
                # full 1_X K0 / 1_Y K0 row sums.  ONE DMA ships it all. ----
                sC = scr.tile([128, W_SC], F32, name="sC", tag="sC")
                nc.vector.scalar_tensor_tensor(
                    out=sC[:, 0:NM], in0=ps_m[:], scalar=1.0,
                    in1=astk[:, 0:NM], op0=ALU.mult, op1=ALU.mult,
                    accum_out=sC[:, NM:NM + 1])
                nc.sync.dma_start(out=sc_d[:], in_=sC[:])

    nc.compile()
    return nc


def _host_prep(X, Y, bandwidths, perms):
    import ml_dtypes

    X = np.ascontiguousarray(X, np.float32)
    Y = np.ascontiguousarray(Y, np.float32)
    perms = np.ascontiguousarray(perms, np.int32)
    Zbf = np.concatenate([X, Y], 0).astype(ml_dtypes.bfloat16)
    Z = Zbf.astype(np.float32)                      # device-visible values
    sq = np.einsum("ij,ij->i", Z, Z, dtype=np.float32).astype(np.float32)
    sq_hi = sq.astype(ml_dtypes.bfloat16)
    sq_lo = (sq - sq_hi.astype(np.float32)).astype(ml_dtypes.bfloat16)

    lrz = np.zeros((66, 2 * NM), ml_dtypes.bfloat16)
    lrz[0:D, 0:NM] = Z.T
    lrz[D:66, 0:NM] = 1.0
    lrz[0:D, NM:] = (-2.0 * Z.T).astype(ml_dtypes.bfloat16)
    lrz[D, NM:] = sq_hi
    lrz[D + 1, NM:] = sq_lo

    b = np.asarray(bandwidths, np.float64)
    scl4 = np.zeros(4, np.float32)
    for k, kern in enumerate(KERNELS):
        scl4[k] = -1.0 / (b[k] * b[k]) if kern == "gaussian" else -1.0 / b[k]

    maps = []
    for cid in range(NC):
        pair, blk = divmod(cid, 4)
        pm = perms[blk * CPP:(blk + 1) * CPP]
        A = np.zeros((52, NM), np.float32)
        A[np.arange(CPP)[:, None], pm[:, :N]] = 1
        A[50, :N] = 1
        A[51, N:] = 1
        astk = np.zeros((128, NM + 16), np.float32)
        atp = np.zeros((128, 6 * 64), np.float32)
        for h in range(2):
            astk[64 * h:64 * h + 52, 0:NM] = A
            astk[64 * h + 50:64 * h + 52, 0:NM] = 1.0  # ship full rows
        for c in range(6):
            atp[:, 64 * c:64 * c + 52] = A[:, 128 * c:128 * (c + 1)].T
        for r in range(6):
            astk[:, NM + r] = sq[128 * r:128 * (r + 1)] + np.float32(BIAS)
        astk[:, NM + 6] = scl4[2 * pair]      # gaussian scale
        astk[:, NM + 7] = scl4[2 * pair + 1]  # laplacian scale
        maps.append(dict(lrz=lrz, atp=atp.astype(ml_dtypes.bfloat16),
                         astk=astk))
    return maps


def _postprocess(results, X, Y, bandwidths, perms):
    """Assemble [4, 1+200] from per-core sC blocks + exact host terms."""
    import ml_dtypes

    X = np.ascontiguousarray(X, np.float32)
    Y = np.ascontiguousarray(Y, np.float32)
    perms = np.asarray(perms, np.int64)
    Z = np.concatenate([X, Y], 0).astype(ml_dtypes.bfloat16).astype(
        np.float64)
    sq = (Z * Z).sum(1)
    b = np.asarray(bandwidths, np.float64)
    j = np.arange(N)
    d2e = sq[j] + sq[N + j] - 2.0 * np.einsum("ij,ij->i", Z[j], Z[N + j])

    def kexp(k, d2):
        d2 = d2 + BIAS
        if KERNELS[k] == "gaussian":
            return np.exp(-d2 / (b[k] * b[k]))
        return np.exp(-np.sqrt(d2) / b[k])

    full = np.zeros((4, 1 + NPER), np.float64)
    for cid in range(NC):
        pair, blk = divmod(cid, 4)
        sc = results[cid]["sc"].astype(np.float64)
        pm = perms[blk * CPP:(blk + 1) * CPP]
        pX, pY = pm[:, :N], pm[:, N:]
        d2p = (sq[pX] + sq[pY]
               - 2.0 * np.einsum("pij,pij->pi", Z[pX], Z[pY]))  # [50, 384]
        stripe = pY == pX + N
        Am = np.zeros((CPP, NM))
        Am[np.arange(CPP)[:, None], pX] = 1.0
        A1, A2 = Am[:, :N], Am[:, N:]
        Wc = -KAP * (A1 * A2) + CB1 * A1 + CB2 * A2
        for h in range(2):
            k = 2 * pair + h
            rsx = sc[64 * h + 50, 0:NM]
            rsy = sc[64 * h + 51, 0:NM]
            q0 = sc[64 * h:64 * h + CPP, NM]
            rs = rsx + rsy
            arow = rs[pX].sum(1)
            ek = kexp(k, d2e)
            Kp = kexp(k, d2p)
            Kp[stripe] = 0.0
            t = Kp.sum(1)
            corr = Wc @ ek
            XX = rsx[:N].sum()
            YX = rsy[:N].sum()
            XY0 = rsx[N:].sum()
            YY = rsy[N:].sum()
            S = XX + YX + XY0 + YY
            sev = ek.sum()
            if KERNELS[k] == "gaussian":
                d0c = np.exp(-BIAS / (b[k] * b[k]))
            else:
                d0c = np.exp(-np.sqrt(BIAS) / b[k])
            tr = 768.0 * d0c
            ck = (S - sev - tr) * IC1
            ub = KAP * (q0 - arow) + corr + TCO * t + ck
            full[k, 1 + blk * CPP:1 + (blk + 1) * CPP] = ub
            if blk == 0:
                full[k, 0] = ((XX + YY - tr) * IC1
                              - 2.0 * IC2 * (XY0 - sev))
    return full.astype(np.float32)


_NC_CACHE = None


def _get_nc():
    global _NC_CACHE
    if _NC_CACHE is None:
        _NC_CACHE = _build()
    return _NC_CACHE


def kernel(X, Y, bandwidths, perms):
    nc = _get_nc()
    in_maps = _host_prep(X, Y, bandwidths, perms)
    res = bass_utils.run_bass_kernel_spmd(nc, in_maps, list(range(NC)))
    return _postprocess(res.results, X, Y, bandwidths, perms)


# revision 22
# speedup vs baseline: 1.4845x; 1.0123x over previous
"""Trainium2 Bass kernel for the 4-kernel MMD permutation test (nn_DUAL_78237124264373).

Sharding: 8 cores x (kernel-pair, perm-block).  Cores 0-3 compute kernels
{0 gaussian, 1 laplacian} for perms 50c..50c+49; cores 4-7 compute kernels
{2, 3} likewise.  Every core runs the SAME program: one wide exp over d2sb
(gaussian) + one over dist (laplacian), so exp work per core is 2x4608
columns instead of 4x.  Device does the O(768^2) work; host the O(768) work.

Per core:
  Z = [X; Y] (768 x 64) is bf16-rounded on host.  d2 row-tiles come from one
  rank-66 bf16 matmul  d2 = L^T R  with L = [Zt; 1; 1], R = [-2 Zt; sq_hi;
  sq_lo]  (sq split hi+lo keeps the diagonal residual ~1e-3).  The ACT sqrt
  reads each PSUM tile directly with per-partition bias sq_r + B (B=1e-2
  keeps the diagonal sqrt finite; host constants match).  After ONE Exp
  table switch (the Sqrt load hides in the DMA wait) each kernel matrix K0
  is a single wide 4608-col bf16 exp, immediately consumed by M0 = A_aug K0
  (A_aug = 50 perm indicators + 1_X + 1_Y, group h at partitions 64h+i).
  One DVE pass makes sC = M0 o A_aug with rowwise accum q0 = a K0 a into
  sC[:, 768]; astk rows 64h+{50,51} are all-ones so those sC rows are the
  unmasked 1_X K0 / 1_Y K0 row sums.  ONE DMA ships sC; the host assembles
  U_b = KAP*(q0 - arow) + W_corr @ e + (2/C2) t + ck and the U column from
  those sums and its own exact pair/stripe exps.
"""

import sys

import numpy as np

if "/opt/trn_rl_repo" not in sys.path:
    sys.path.insert(0, "/opt/trn_rl_repo")

import concourse.bacc as bacc
import concourse.bass as bass
import concourse.mybir as mybir
import concourse.tile as tile
from concourse import bass_utils

N = 384
NM = 768
D = 64
NPER = 200
NC = 8
CPP = 50   # perms per core (4 perm-blocks x 2 kernel-pairs)
C1 = float(N * (N - 1))
C2 = float(N * N)
KAP = 2.0 / C1 + 2.0 / C2
CB1 = 1.0 / C1 + 2.0 / C2
CB2 = 1.0 / C1
TCO = 2.0 / C2
IC1 = 1.0 / C1
IC2 = 1.0 / C2
BIAS = 0.01  # added under the laplacian sqrt; host terms match
KERNELS = ("gaussian", "laplacian", "gaussian", "laplacian")

F32 = mybir.dt.float32
BF16 = mybir.dt.bfloat16
AF = mybir.ActivationFunctionType
ALU = mybir.AluOpType

W_SC = 770  # sC: masked M0 cols 0:768, q0 partial-accum cols 768/769


def _build():
    nc = bacc.Bacc("TRN2", target_bir_lowering=False, debug=False)
    with tile.TileContext(nc) as tc:
        with tc.tile_pool(name="dram", bufs=1, space="DRAM") as dram, \
             tc.tile_pool(name="io", bufs=1) as io, \
             tc.tile_pool(name="big", bufs=1) as big, \
             tc.tile_pool(name="kpool", bufs=2) as kpool, \
             tc.tile_pool(name="scr", bufs=1) as scr, \
             tc.tile_pool(name="sml", bufs=1) as sml:

            def din(name, shape, dt=F32):
                return dram.tile(shape, dt, kind="ExternalInput", name=name,
                                 uniquify=False)

            lrz_d = din("lrz", [66, 2 * NM], BF16)
            atp_d = din("atp", [128, 6 * 64], BF16)
            astk_d = din("astk", [128, NM + 16])
            sc_d = dram.tile([128, W_SC], F32, kind="ExternalOutput",
                             name="sc", uniquify=False)

            # ---- input DMAs (all on sync; lrz first) ----
            lrz = io.tile([66, 2 * NM], BF16, name="lrz_sb")
            atp = io.tile([128, 6 * 64], BF16, name="atp_sb")
            astk = io.tile([128, NM + 16], F32, name="astk_sb")
            nc.sync.dma_start(out=lrz[:], in_=lrz_d[:])
            nc.sync.dma_start(out=astk[:, NM:], in_=astk_d[:, NM:])
            nc.sync.dma_start(out=atp[:], in_=atp_d[:])
            nc.sync.dma_start(out=astk[:, 0:NM], in_=astk_d[:, 0:NM])

            L = lrz[:, 0:NM]
            R = lrz[:, NM:2 * NM]
            sqcB = astk[:, NM:NM + 6]
            scl = astk[:, NM + 6:NM + 8]   # col 0: gaussian, col 1: laplacian

            d2sb = big.tile([128, 6 * NM], F32, name="d2sb")
            dist = big.tile([128, 6 * NM], F32, name="dist_sb")
            ones = io.tile([128, 1], F32, name="ones_sb")
            nc.vector.memset(ones[:], 1.0)

            # Dummy sqrt: hides the Sqrt ACT-table load in the DMA wait.
            dummy = sml.tile([1, 1], F32, name="dummy")
            nc.scalar.activation(dummy[:], ones[0:1, 0:1], AF.Sqrt)

            kts = [kpool.tile([128, 6 * NM], BF16, name=f"kt{h}",
                              tag="kt") for h in range(2)]

            # ---- phase 1: d2 = L^T R per row-tile (4 rotating PSUM bufs);
            # ACT sqrt straight out of PSUM with bias sq_r + B, DVE adds the
            # same bias into f32 d2sb for the gaussian exp ----
            with tc.tile_pool(name="psA", bufs=4, space="PSUM") as psA:
                for r in range(6):
                    ps = psA.tile([128, NM], F32, name=f"d2t{r}", tag="d2")
                    lhs = L[:, 128 * r:128 * (r + 1)]
                    nc.tensor.matmul(ps[:, 0:512], lhs, R[:, 0:512],
                                     start=True, stop=True)
                    nc.tensor.matmul(ps[:, 512:768], lhs, R[:, 512:768],
                                     start=True, stop=True)
                    sl = slice(NM * r, NM * (r + 1))
                    nc.scalar.activation(dist[:, sl], ps[:], AF.Sqrt,
                                         bias=sqcB[:, r:r + 1])
                    nc.vector.tensor_scalar(
                        out=d2sb[:, sl], in0=ps[:],
                        scalar1=sqcB[:, r:r + 1], scalar2=0.0,
                        op0=ALU.add, op1=ALU.add)

            with tc.tile_pool(name="psB", bufs=1, space="PSUM") as psB:
                ps_m = psB.tile([128, NM], F32, name="ps_m")

                # ---- phase 2: one wide exp per kernel, then its M0 ----
                def m0_chunks(h, cs):
                    pr = slice(64 * h, 64 * h + 64)
                    for c in cs:
                        lhs = atp[:, 64 * c:64 * (c + 1)]
                        nc.tensor.matmul(ps_m[pr, 0:512], lhs,
                                         kts[h][:, NM * c:NM * c + 512],
                                         start=(c == 0), stop=(c == 5),
                                         tile_position=(0, 64 * h),
                                         skip_group_check=True)
                        nc.tensor.matmul(ps_m[pr, 512:NM], lhs,
                                         kts[h][:, NM * c + 512:NM * (c + 1)],
                                         start=(c == 0), stop=(c == 5),
                                         tile_position=(0, 64 * h),
                                         skip_group_check=True)

                # laplacian first (dist is ready right after the sqrts);
                # gaussian in halves so its M0 overlaps the second half
                nc.scalar.activation(kts[1][:], dist[:], AF.Exp,
                                     scale=scl[:, 1:2])
                m0_chunks(1, range(6))
                nc.scalar.activation(kts[0][:, 0:3 * NM], d2sb[:, 0:3 * NM],
                                     AF.Exp, scale=scl[:, 0:1])
                m0_chunks(0, range(3))
                nc.scalar.activation(kts[0][:, 3 * NM:6 * NM],
                                     d2sb[:, 3 * NM:6 * NM],
                                     AF.Exp, scale=scl[:, 0:1])
                m0_chunks(0, range(3, 6))

                # ---- phase 3: sC = M0 o A_aug in two column halves, each
                # with its own q0 partial-accum column (host sums them);
                # astk rows 64h+{50,51} are ones so those rows ship the
                # full 1_X K0 / 1_Y K0 row sums.  Two DMAs so the first
                # wake overlaps the second half-pass. ----
                sC = scr.tile([128, W_SC], F32, name="sC", tag="sC")
                nc.vector.scalar_tensor_tensor(
                    out=sC[:, 0:N], in0=ps_m[:, 0:N], scalar=1.0,
                    in1=astk[:, 0:N], op0=ALU.mult, op1=ALU.mult,
                    accum_out=sC[:, NM:NM + 1])
                nc.sync.dma_start(out=sc_d[:, 0:N], in_=sC[:, 0:N])
                nc.vector.scalar_tensor_tensor(
                    out=sC[:, N:NM], in0=ps_m[:, N:NM], scalar=1.0,
                    in1=astk[:, N:NM], op0=ALU.mult, op1=ALU.mult,
                    accum_out=sC[:, NM + 1:NM + 2])
                nc.sync.dma_start(out=sc_d[:, N:W_SC], in_=sC[:, N:W_SC])

    nc.compile()
    return nc


def _host_prep(X, Y, bandwidths, perms):
    import ml_dtypes

    X = np.ascontiguousarray(X, np.float32)
    Y = np.ascontiguousarray(Y, np.float32)
    perms = np.ascontiguousarray(perms, np.int32)
    Zbf = np.concatenate([X, Y], 0).astype(ml_dtypes.bfloat16)
    Z = Zbf.astype(np.float32)                      # device-visible values
    sq = np.einsum("ij,ij->i", Z, Z, dtype=np.float32).astype(np.float32)
    sq_hi = sq.astype(ml_dtypes.bfloat16)
    sq_lo = (sq - sq_hi.astype(np.float32)).astype(ml_dtypes.bfloat16)

    lrz = np.zeros((66, 2 * NM), ml_dtypes.bfloat16)
    lrz[0:D, 0:NM] = Z.T
    lrz[D:66, 0:NM] = 1.0
    lrz[0:D, NM:] = (-2.0 * Z.T).astype(ml_dtypes.bfloat16)
    lrz[D, NM:] = sq_hi
    lrz[D + 1, NM:] = sq_lo

    b = np.asarray(bandwidths, np.float64)
    scl4 = np.zeros(4, np.float32)
    for k, kern in enumerate(KERNELS):
        scl4[k] = -1.0 / (b[k] * b[k]) if kern == "gaussian" else -1.0 / b[k]

    maps = []
    for cid in range(NC):
        pair, blk = divmod(cid, 4)
        pm = perms[blk * CPP:(blk + 1) * CPP]
        A = np.zeros((52, NM), np.float32)
        A[np.arange(CPP)[:, None], pm[:, :N]] = 1
        A[50, :N] = 1
        A[51, N:] = 1
        astk = np.zeros((128, NM + 16), np.float32)
        atp = np.zeros((128, 6 * 64), np.float32)
        for h in range(2):
            astk[64 * h:64 * h + 52, 0:NM] = A
            astk[64 * h + 50:64 * h + 52, 0:NM] = 1.0  # ship full rows
        for c in range(6):
            atp[:, 64 * c:64 * c + 52] = A[:, 128 * c:128 * (c + 1)].T
        for r in range(6):
            astk[:, NM + r] = sq[128 * r:128 * (r + 1)] + np.float32(BIAS)
        astk[:, NM + 6] = scl4[2 * pair]      # gaussian scale
        astk[:, NM + 7] = scl4[2 * pair + 1]  # laplacian scale
        maps.append(dict(lrz=lrz, atp=atp.astype(ml_dtypes.bfloat16),
                         astk=astk))
    return maps


def _postprocess(results, X, Y, bandwidths, perms):
    """Assemble [4, 1+200] from per-core sC blocks + exact host terms."""
    import ml_dtypes

    X = np.ascontiguousarray(X, np.float32)
    Y = np.ascontiguousarray(Y, np.float32)
    perms = np.asarray(perms, np.int64)
    Z = np.concatenate([X, Y], 0).astype(ml_dtypes.bfloat16).astype(
        np.float64)
    sq = (Z * Z).sum(1)
    b = np.asarray(bandwidths, np.float64)
    j = np.arange(N)
    d2e = sq[j] + sq[N + j] - 2.0 * np.einsum("ij,ij->i", Z[j], Z[N + j])

    def kexp(k, d2):
        d2 = d2 + BIAS
        if KERNELS[k] == "gaussian":
            return np.exp(-d2 / (b[k] * b[k]))
        return np.exp(-np.sqrt(d2) / b[k])

    full = np.zeros((4, 1 + NPER), np.float64)
    for cid in range(NC):
        pair, blk = divmod(cid, 4)
        sc = results[cid]["sc"].astype(np.float64)
        pm = perms[blk * CPP:(blk + 1) * CPP]
        pX, pY = pm[:, :N], pm[:, N:]
        d2p = (sq[pX] + sq[pY]
               - 2.0 * np.einsum("pij,pij->pi", Z[pX], Z[pY]))  # [50, 384]
        stripe = pY == pX + N
        Am = np.zeros((CPP, NM))
        Am[np.arange(CPP)[:, None], pX] = 1.0
        A1, A2 = Am[:, :N], Am[:, N:]
        Wc = -KAP * (A1 * A2) + CB1 * A1 + CB2 * A2
        for h in range(2):
            k = 2 * pair + h
            rsx = sc[64 * h + 50, 0:NM]
            rsy = sc[64 * h + 51, 0:NM]
            q0 = sc[64 * h:64 * h + CPP, NM:NM + 2].sum(1)
            rs = rsx + rsy
            arow = rs[pX].sum(1)
            ek = kexp(k, d2e)
            Kp = kexp(k, d2p)
            Kp[stripe] = 0.0
            t = Kp.sum(1)
            corr = Wc @ ek
            XX = rsx[:N].sum()
            YX = rsy[:N].sum()
            XY0 = rsx[N:].sum()
            YY = rsy[N:].sum()
            S = XX + YX + XY0 + YY
            sev = ek.sum()
            if KERNELS[k] == "gaussian":
                d0c = np.exp(-BIAS / (b[k] * b[k]))
            else:
                d0c = np.exp(-np.sqrt(BIAS) / b[k])
            tr = 768.0 * d0c
            ck = (S - sev - tr) * IC1
            ub = KAP * (q0 - arow) + corr + TCO * t + ck
            full[k, 1 + blk * CPP:1 + (blk + 1) * CPP] = ub
            if blk == 0:
                full[k, 0] = ((XX + YY - tr) * IC1
                              - 2.0 * IC2 * (XY0 - sev))
    return full.astype(np.float32)


_NC_CACHE = None


def _get_nc():
    global _NC_CACHE
    if _NC_CACHE is None:
        _NC_CACHE = _build()
    return _NC_CACHE


def kernel(X, Y, bandwidths, perms):
    nc = _get_nc()
    in_maps = _host_prep(X, Y, bandwidths, perms)
    res = bass_utils.run_bass_kernel_spmd(nc, in_maps, list(range(NC)))
    return _postprocess(res.results, X, Y, bandwidths, perms)
